# revision 7
# baseline (speedup 1.0000x reference)
"""Trainium2 Bass kernel for nn_Attention_62938450756123.

Reference computation (per batch b):
    oe[s, h] = out_e[s, b, 0:512] + out_e[s, b, 512:1024]      # bidirectional sum
    od[t, h] = out_d[t, b, :]
    S[s, t]  = sum_h oe[s, h] * od[t, h]
    p[s, t]  = exp(S[s, t])                                     # naive, no max-sub
    ctx[t,h] = (sum_s p[s, t] * oe[s, h]) / (sum_s p[s, t])
    out[t, b, h] = ctx[t, h]

Sharding: data-parallel over batch (bs=16) across 8 NeuronCores, 2 batches
per core, no collectives.

Per-core dataflow (all matmuls bf16 on TensorE, f32 PSUM accumulate):
  - GPSIMD (SWDGE) cast-loads f32->bf16: out_e halves + out_d tiles.
  - VectorE sums the out_e halves -> oe tiles bf16 [s128, h512].
  - h-major layouts are built ON TensorE: for each 128x128 block,
    psum[h, s'] = sum_s x[s, h] * I[s, s']  (normal matmul, identity moving).
    Four h-chunks pack into one PSUM bank; one VectorE copy moves the bank
    to SBUF bf16: oeT_i [128p, 4hc, 128s], odT_chunk [128p, 4hc, 512t]
    (h = hc*128 + p).  DMA-xbar transposes are NOT used: Tile serializes
    them against every other DMA (HW-deadlock workaround), which makes the
    whole load stream ping-pong at ~6us per transfer.
  - mm1: psum_S[s128, t512] = sum_hc oeT_i.T @ odT ; exp on ScalarE -> P bf16
  - mm2: psum_ctx[t128, h512] += P_i.T @ oe_i ; psum_den[t128, 1] +=
    P_i.T @ ones   (same stationary weights, +25ns/pair measured)
  - normalize on VectorE (reciprocal + tensor_scalar), store via Sync HWDGE.

Buffers are allocated per-s-tile (separate Tile objects) so dependency
tracking stays precise.
"""

import ml_dtypes
import numpy as np

import concourse.bass as bass
import concourse.tile as tile
from concourse import bacc, mybir
from concourse.bass_utils import run_bass_kernel_spmd

SL, TL, BS, H = 2048, 2048, 16, 512
NCORES = 8
BPC = BS // NCORES  # batches per core

F32 = mybir.dt.float32
BF16 = mybir.dt.bfloat16

NS = SL // 128        # 16 s-tiles
NH = H // 128         # 4 h-chunks
TCHUNK = 512          # t-chunk (one PSUM bank of f32)
NTC = TL // TCHUNK    # 4 t-chunks
TPC = TCHUNK // 128   # 4 t-tiles per chunk


def build():
    nc = bacc.Bacc("TRN2", target_bir_lowering=False, debug=False,
                   num_devices=NCORES)
    out_e = nc.dram_tensor("out_e", [SL, BPC, 2 * H], F32,
                           kind="ExternalInput").ap()
    out_d = nc.dram_tensor("out_d", [TL, BPC, H], F32,
                           kind="ExternalInput").ap()
    ident = nc.dram_tensor("ident", [128, 128], BF16,
                           kind="ExternalInput").ap()
    out = nc.dram_tensor("out", [TL, BPC, H], F32,
                         kind="ExternalOutput").ap()

    exp = mybir.ActivationFunctionType.Exp

    with tile.TileContext(nc) as tc:
        with (
            tc.tile_pool(name="consts", bufs=1) as consts,
            tc.tile_pool(name="stage_e", bufs=8) as stage_e_pool,
            tc.tile_pool(name="oenat", bufs=2 * NS) as oenat_pool,
            tc.tile_pool(name="odnat", bufs=8) as odnat_pool,
            tc.tile_pool(name="oet", bufs=2 * NS) as oet_pool,
            tc.tile_pool(name="odt", bufs=2 * NTC) as odt_pool,
            tc.tile_pool(name="pbuf", bufs=2 * NS) as p_pool,
            tc.tile_pool(name="osb", bufs=3) as osb_pool,
            tc.tile_pool(name="small", bufs=4) as small_pool,
            tc.tile_pool(name="psS", bufs=3, space="PSUM") as psS_pool,
            tc.tile_pool(name="psC", bufs=2, space="PSUM") as psC_pool,
            tc.tile_pool(name="psD", bufs=1, space="PSUM") as psD_pool,
            tc.tile_pool(name="ptr", bufs=2, space="PSUM") as ptr_pool,
        ):
            ones = consts.tile([128, 1], BF16, tag="ones")
            nc.vector.memset(ones, 1.0)
            idt = consts.tile([128, 128], BF16, tag="idt")
            nc.sync.dma_start(idt, ident)

            def transpose_tiles(src, dst, n, copy_engine):
                """src [128, n*128] bf16 -> dst [128, n, 128] with
                dst[p, c, j] = src[j, c*128 + p], via n identity matmuls
                packed into one PSUM bank + one copy to SBUF."""
                pt = ptr_pool.tile([128, n * 128], F32, tag="ptr")
                for c in range(n):
                    nc.tensor.matmul(pt[:, c * 128:(c + 1) * 128],
                                     src[:, c * 128:(c + 1) * 128], idt,
                                     start=True, stop=True)
                if copy_engine == "scalar":
                    nc.scalar.copy(dst, pt)
                else:
                    nc.vector.tensor_copy(dst, pt)

            for b in range(BPC):
                # per-s-tile buffers for this batch
                oe_tiles = []   # [128, H] bf16, natural layout
                oeT_tiles = []  # [128, NH, 128] bf16, h-major
                odT_chunks = []  # [128, NH, TCHUNK] bf16, h-major

                def load_d(i):
                    # HWDGE f32 load + DVE cast: od traffic rides the
                    # otherwise-idle Sync queue, in parallel with the
                    # GPSIMD out_e stream.  4 od t-tiles feed one odT chunk.
                    ci, k = divmod(i, TPC)
                    if k == 0:
                        odc = odt_pool.tile([128, NH, TCHUNK], BF16, tag="odT",
                                            name=f"odT_{b}_{ci}")
                        odT_chunks.append(odc)
                    odc = odT_chunks[ci]
                    sdf = odnat_pool.tile([128, H], F32, tag="odf",
                                          name=f"odf_{b}_{i}")
                    sd = odnat_pool.tile([128, H], BF16, tag="od",
                                         name=f"od_{b}_{i}")
                    nc.sync.dma_start(sdf, out_d[i * 128:(i + 1) * 128, b, :])
                    nc.vector.tensor_copy(sd, sdf)
                    transpose_tiles(sd, odc[:, :, k * 128:(k + 1) * 128], NH,
                                    "scalar")

                def load_e(i):
                    oe = oenat_pool.tile([128, H], BF16, tag="oe",
                                         name=f"oe_{b}_{i}")
                    oeT = oet_pool.tile([128, NH, 128], BF16, tag="oeT",
                                        name=f"oeT_{b}_{i}")
                    oe_tiles.append(oe)
                    oeT_tiles.append(oeT)
                    st = stage_e_pool.tile([128, 2 * H], BF16, tag="st",
                                           name=f"st_{b}_{i}")
                    # single SWDGE cast-load brings both halves
                    nc.gpsimd.dma_start(st, out_e[i * 128:(i + 1) * 128, b, :])
                    nc.vector.tensor_add(oe, st[:, 0:H], st[:, H:2 * H])
                    transpose_tiles(oe, oeT, NH, "vector")

                def mm1(tci, i, P_tiles):
                    psS = psS_pool.tile([128, TCHUNK], F32, tag="psS")
                    for c in range(NH):
                        nc.tensor.matmul(
                            psS,
                            oeT_tiles[i][:, c, :],
                            odT_chunks[tci][:, c, :],
                            start=(c == 0), stop=(c == NH - 1))
                    P = p_pool.tile([128, TCHUNK], BF16, tag="P",
                                    name=f"P_{b}_{tci}_{i}")
                    P_tiles.append(P)
                    nc.scalar.activation(P, psS, exp)

                # Pipeline batch preprocessing with chunk-0 mm1: transposes
                # for s-tile i are followed by mm1 on s-tile i-1 (one tile of
                # lag hides the PE->DVE->PE round trip through oeT).
                P0_tiles = []
                for i in range(TPC):
                    load_d(i)
                for i in range(NS):
                    load_e(i)
                    if TPC + i < NS:
                        load_d(TPC + i)
                    if i >= 1:
                        mm1(0, i - 1, P0_tiles)
                mm1(0, NS - 1, P0_tiles)

                for tci in range(NTC):
                    P_tiles = P0_tiles if tci == 0 else []
                    if tci > 0:
                        for i in range(NS):
                            mm1(tci, i, P_tiles)
                    for tt in range(TPC):
                        psC = psC_pool.tile([128, H], F32, tag="psC")
                        psD = psD_pool.tile([128, 1], F32, tag="psD")
                        for i in range(NS):
                            lhsT = P_tiles[i][:, tt * 128:(tt + 1) * 128]
                            nc.tensor.matmul(psC, lhsT, oe_tiles[i],
                                             start=(i == 0), stop=(i == NS - 1))
                            nc.tensor.matmul(psD, lhsT, ones,
                                             start=(i == 0), stop=(i == NS - 1))
                        rc = small_pool.tile([128, 1], F32, tag="rc")
                        nc.vector.reciprocal(rc, psD)
                        ob = osb_pool.tile([128, H], F32, tag="ob")
                        nc.vector.tensor_scalar(ob, psC, rc, None,
                                                mybir.AluOpType.mult)
                        t0 = tci * TCHUNK + tt * 128
                        nc.sync.dma_start(out[t0:t0 + 128, b, :], ob)

    nc.compile()
    return nc


_nc = None
last_result = None
_IDENT = np.eye(128).astype(ml_dtypes.bfloat16)


def kernel(in_e=None, out_e=None, out_d=None, _trace=False, **_unused):
    global _nc, last_result
    if _nc is None:
        _nc = build()
    out_e = np.asarray(out_e, dtype=np.float32)
    out_d = np.asarray(out_d, dtype=np.float32)
    in_maps = []
    for c in range(NCORES):
        sl = slice(c * BPC, (c + 1) * BPC)
        in_maps.append({
            "out_e": np.ascontiguousarray(out_e[:, sl, :]),
            "out_d": np.ascontiguousarray(out_d[:, sl, :]),
            "ident": _IDENT,
        })
    last_result = run_bass_kernel_spmd(_nc, in_maps,
                                       core_ids=list(range(NCORES)),
                                       trace=_trace)
    return np.concatenate(
        [np.asarray(last_result.results[c]["out"]) for c in range(NCORES)],
        axis=1).astype(np.float32)
